# revision 15
# baseline (speedup 1.0000x reference)
"""Trainium2 Bass kernel for nn_MessageFunction (gnn_message_passing).

Computes, per edge e:
    x  = relu(e_vw @ W0.T + b0)                 # [E, 128]
    x  = relu(x @ W1.T + b1)                    # [E, 128]
    eo = (x @ W2.T + b2).reshape(E, 32, 32)     # [E, o, i]
    m  = einsum('eoi,ei->eo', eo, h_w)          # [E, 32]

Sharding: pure edge parallelism across 8 NeuronCores (E/8 = 16384 edges per
core), NNet parameters replicated.

Per-core layout strategy (host-side pre-transposition, fp16 on-chip matmul
dtypes, fp32 PSUM accumulation), supertiles of 512 edges:
  - L0/L1 run feature-major (hidden on partitions, edges free); relu+bias
    evictions on the scalar engine.
  - L2 runs oi-major: 4 pairs of 128-row chunks; pair p covers o in
    [8p, 8p+8) with i split in halves between its two chunks (W2 columns
    pair-reordered on host). Each pair lands in one [128, 2, 512] PSUM tile.
  - The per-edge h_w multiply is a fused PSUM-evict+multiply
    (scalar_tensor_tensor) on the vector engine for half the pairs, and a
    scalar-engine copy-evict + 2x-mode tensor_mul for the other half.
  - The i-contraction uses PE column-tiling: pair p's two selection matmuls
    (one shared [128, 8] 0/1 matrix) accumulate into col-strip p
    (tile_position (0, 32p)) of a single [128, 512] PSUM tile, so the four
    strips run concurrently on disjoint 32-column groups of the PE array.
    Each o lives in exactly one strip row -> no cross-strip reduction.
  - The b2 term (sum_i b2[o,i] h_w[e,i]) is independent of the NNet and is
    added on the host (cheap [E,32]@[32,32] einsum) during unsharding.
"""

import os
import sys
from contextlib import ExitStack

import numpy as np

sys.path.insert(0, "/opt/trn_rl_repo")

import concourse.bass as bass
import concourse.tile as tile
from concourse import bacc, mybir
from concourse._compat import with_exitstack
from concourse.bass_utils import run_bass_kernel_spmd

E = 131072
N_CORES = 8
E_CORE = E // N_CORES          # 16384
TILE_E = 128                   # edges per PE tile (e-major matmul M)
SUPER = 4                      # tiles per supertile
SUPER_E = SUPER * TILE_E       # 512
N_SUPER = E_CORE // SUPER_E    # 32
HID = 128
EF = 16
D = 32                         # D_IN == D_OUT == 32
OI = D * D                     # 1024
N_CHUNK = OI // 128            # 8
N_PAIR = N_CHUNK // 2          # 4

F32 = mybir.dt.float32
F16 = mybir.dt.float16

# Pairs [0, N_STT) evict+multiply fused on the DVE; the rest use an ACT
# copy-evict followed by a 2x-mode DVE multiply. 2/2 balances DVE vs ACT.
N_STT = 2


@with_exitstack
def _edge_mlp_kernel(
    ctx: ExitStack,
    tc: "tile.TileContext",
    out_mT: bass.AP,      # [32, E_CORE] fp32, o-major output (no b2 term)
    ev_t: bass.AP,        # [N_SUPER, EF, SUPER_E] fp16  (e_vw transposed)
    hw8: bass.AP,         # [N_SUPER, 128, 2, SUPER_E] fp16 (h_w^T in pair layout)
    w0t: bass.AP,         # [EF, HID] fp16
    w1t: bass.AP,         # [HID, HID] fp16
    w2t: bass.AP,         # [HID, OI] fp16 (columns pair-reordered on host)
    scm: bass.AP,         # [128, 8] fp16 shared selection matrix
    b0: bass.AP,          # [HID, 1] fp32
    b1: bass.AP,          # [HID, 1] fp32
):
    nc = tc.nc
    Relu = mybir.ActivationFunctionType.Relu
    Copy = mybir.ActivationFunctionType.Copy

    const = ctx.enter_context(tc.tile_pool(name="const", bufs=1))
    sup = ctx.enter_context(tc.tile_pool(name="sup", bufs=6))
    ypool = ctx.enter_context(tc.tile_pool(name="y", bufs=4))
    opool = ctx.enter_context(tc.tile_pool(name="o", bufs=3))
    ps_x = ctx.enter_context(tc.tile_pool(name="psx", bufs=1, space="PSUM"))
    ps_eo = ctx.enter_context(tc.tile_pool(name="pseo", bufs=3, space="PSUM"))
    ps_p = ctx.enter_context(tc.tile_pool(name="psp", bufs=1, space="PSUM"))

    # --- load constants once ---
    c_w0 = const.tile([EF, HID], F16)
    nc.sync.dma_start(c_w0[:], w0t[:])
    c_w1 = const.tile([HID, HID], F16)
    nc.sync.dma_start(c_w1[:], w1t[:])
    c_w2 = const.tile([HID, OI], F16)
    nc.sync.dma_start(c_w2[:], w2t[:])
    c_sc = const.tile([128, 8], F16)
    nc.sync.dma_start(c_sc[:], scm[:])
    c_b0 = const.tile([HID, 1], F32)
    nc.sync.dma_start(c_b0[:], b0[:])
    c_b1 = const.tile([HID, 1], F32)
    nc.sync.dma_start(c_b1[:], b1[:])

    # PE warmup: ~4us of matmuls on zeroed data while the first supertile's
    # DMAs land, so the HAM clock-gate reaches 8/8 before the real matmuls
    # start. Results land in the partials PSUM slot that the first real
    # start=True matmul group overwrites.
    warm_in = const.tile([128, SUPER_E], F16, tag="warm")
    warm_ps = ps_p.tile([128, SUPER_E], F32, tag="pp")
    nc.gpsimd.memset(warm_in[:], 0.0)
    for _ in range(18):
        nc.tensor.matmul(warm_ps[:], warm_in[:, 0:128], warm_in[:])

    def emit_sel(s, pp, ys):
        # i-contraction for supertile s: accumulating selection matmuls
        # into col-strip p (rows l in [0, 8) hold o = 8p + l). Emitted one
        # supertile late so all four y tiles already exist -> the PE runs
        # the block as two 4-strip concurrent bursts on disjoint column
        # groups instead of eight serial matmuls.
        for j in range(2):
            for p in range(N_PAIR):
                nc.tensor.matmul(
                    pp[32 * p : 32 * p + 8, :], c_sc[:], ys[p][:, j, :],
                    start=(j == 0), stop=(j == 1),
                    tile_position=(0, 32 * p),
                )
        # evict partials and store the 8 useful rows of each strip
        ps = opool.tile([128, SUPER_E], F16, tag="ps")
        nc.scalar.activation(ps[:], pp[:], Copy)
        for p in range(N_PAIR):
            nc.sync.dma_start(
                out_mT[8 * p : 8 * p + 8, s * SUPER_E : (s + 1) * SUPER_E],
                ps[32 * p : 32 * p + 8, :],
            )

    pend = None  # (s, pp, ys) of the previous supertile
    for s in range(N_SUPER):
        ev = sup.tile([EF, SUPER_E], F16, tag="ev")
        nc.sync.dma_start(ev[:], ev_t[s])
        hw = sup.tile([128, 2, SUPER_E], F16, tag="hw")
        nc.sync.dma_start(hw[:], hw8[s])

        # L0: x1T[h, e] = sum_f W0T[f, h] * evT[f, e]
        x1p = ps_x.tile([HID, SUPER_E], F32, tag="xp")
        nc.tensor.matmul(x1p[:], c_w0[:], ev[:])
        x1s = sup.tile([HID, SUPER_E], F16, tag="x1s")
        nc.scalar.activation(x1s[:], x1p[:], Relu, bias=c_b0[:])

        # L1: x2T[h2, e] = sum_h W1T[h, h2] * x1T[h, e]
        x2p = ps_x.tile([HID, SUPER_E], F32, tag="xp")
        nc.tensor.matmul(x2p[:], c_w1[:], x1s[:])
        x2s = sup.tile([HID, SUPER_E], F16, tag="x2s")
        nc.scalar.activation(x2s[:], x2p[:], Relu, bias=c_b1[:])

        # L2 + h_w multiply, per chunk pair.
        pp = ps_p.tile([128, SUPER_E], F32, tag="pp")
        ys = []
        for p in range(N_PAIR):
            c0 = 2 * p
            eo = ps_eo.tile([128, 2, SUPER_E], F32, tag="eo")
            nc.tensor.matmul(eo[:, 0, :], c_w2[:, c0 * 128 : (c0 + 1) * 128], x2s[:])
            nc.tensor.matmul(eo[:, 1, :], c_w2[:, (c0 + 1) * 128 : (c0 + 2) * 128], x2s[:])
            yc = ypool.tile([128, 2, SUPER_E], F16, tag=f"y{p}")
            if p < N_STT:
                # fused evict+mult on DVE: y = eo * hw
                nc.vector.scalar_tensor_tensor(
                    yc[:], eo[:], 1.0, hw[:],
                    op0=mybir.AluOpType.mult, op1=mybir.AluOpType.mult,
                )
            else:
                # ACT evicts (fp32 psum -> fp16 sbuf), DVE multiplies at 2x
                eos = ypool.tile([128, 2, SUPER_E], F16, tag=f"eos{p % 2}")
                nc.scalar.activation(eos[:], eo[:], Copy)
                nc.vector.tensor_mul(yc[:], eos[:], hw[:])
            ys.append(yc)
            # the previous supertile's i-contraction slots in mid-L2, when
            # all four of its y tiles are long since produced
            if p == 1 and pend is not None:
                emit_sel(*pend)
                pend = None
        pend = (s, pp, ys)

    emit_sel(*pend)


def _build_bass():
    nc = bacc.Bacc("TRN2", target_bir_lowering=False, debug=False)
    d = {}
    d["ev_t"] = nc.dram_tensor("ev_t", [N_SUPER, EF, SUPER_E], F16, kind="ExternalInput")
    d["hw8"] = nc.dram_tensor("hw8", [N_SUPER, 128, 2, SUPER_E], F16, kind="ExternalInput")
    d["w0t"] = nc.dram_tensor("w0t", [EF, HID], F16, kind="ExternalInput")
    d["w1t"] = nc.dram_tensor("w1t", [HID, HID], F16, kind="ExternalInput")
    d["w2t"] = nc.dram_tensor("w2t", [HID, OI], F16, kind="ExternalInput")
    d["scm"] = nc.dram_tensor("scm", [128, 8], F16, kind="ExternalInput")
    d["b0"] = nc.dram_tensor("b0", [HID, 1], F32, kind="ExternalInput")
    d["b1"] = nc.dram_tensor("b1", [HID, 1], F32, kind="ExternalInput")
    out = nc.dram_tensor("out_mT", [D, E_CORE], F16, kind="ExternalOutput")

    with tile.TileContext(nc) as tc:
        _edge_mlp_kernel(
            tc,
            out.ap(),
            d["ev_t"].ap(), d["hw8"].ap(),
            d["w0t"].ap(), d["w1t"].ap(), d["w2t"].ap(),
            d["scm"].ap(),
            d["b0"].ap(), d["b1"].ap(),
        )
    nc.compile()
    return nc


def _prep_host_inputs(h_w, e_vw, W0, b0, W1, b1, W2):
    """Build per-core input maps (all numpy, cheap)."""
    # shared (replicated) parameters
    w0t = np.ascontiguousarray(W0.T).astype(np.float16)            # [16, 128]
    w1t = np.ascontiguousarray(W1.T).astype(np.float16)            # [128, 128]
    # W2 columns in pair layout: pair p covers o in [8p, 8p+8); its two
    # chunks take i in [0,16) and [16,32). Within a chunk, partition
    # index = (o - 8p)*16 + (i mod 16).
    cols = np.empty((OI,), np.int64)
    for p in range(4):
        for j in range(2):
            o = np.repeat(np.arange(8 * p, 8 * p + 8), 16)          # [128]
            i = np.tile(np.arange(16 * j, 16 * j + 16), 8)          # [128]
            cols[(2 * p + j) * 128 : (2 * p + j + 1) * 128] = o * D + i
    w2t = np.ascontiguousarray(W2.T[:, cols]).astype(np.float16)    # [128, 1024]
    b0c = np.ascontiguousarray(b0.reshape(HID, 1)).astype(np.float32)
    b1c = np.ascontiguousarray(b1.reshape(HID, 1)).astype(np.float32)
    # shared selection matrix: scm[q, l] = 1 iff l == q // 16
    scm = np.zeros((128, 8), np.float16)
    q = np.arange(128)
    scm[q, q // 16] = 1.0

    in_maps = []
    for core in range(N_CORES):
        sl = slice(core * E_CORE, (core + 1) * E_CORE)
        ev_c = e_vw[sl]                                             # [16384, 16]
        hw_c = h_w[sl]                                              # [16384, 32]
        # ev_t[s, f, t*128+e] = ev_c[s*512 + t*128 + e, f]
        ev_t = np.ascontiguousarray(
            ev_c.reshape(N_SUPER, SUPER_E, EF).transpose(0, 2, 1)
        ).astype(np.float16)
        hw_t = hw_c.reshape(N_SUPER, SUPER_E, D).transpose(0, 2, 1)  # [Ns, 32, 512]
        # hw8[s, q, j, e] = hwT[s, 16*j + q%16, e]
        hw8 = np.empty((N_SUPER, 128, 2, SUPER_E), np.float16)
        qm = np.arange(128) % 16
        hw8[:, :, 0, :] = hw_t[:, qm, :]
        hw8[:, :, 1, :] = hw_t[:, 16 + qm, :]
        in_maps.append({
            "ev_t": ev_t, "hw8": hw8,
            "w0t": w0t, "w1t": w1t, "w2t": w2t,
            "scm": scm, "b0": b0c, "b1": b1c,
        })
    return in_maps


_CACHE = {}


def kernel(h_v, h_w, e_vw, W0, b0, W1, b1, W2, b2, _trace=False, _results=None):
    # h_v is unused by the reference computation (only its trailing dim of 1
    # matters there); the message depends on h_w, e_vw and the NNet params.
    del h_v
    h_w = np.asarray(h_w, np.float32)
    in_maps = _prep_host_inputs(
        h_w, np.asarray(e_vw, np.float32),
        np.asarray(W0, np.float32), np.asarray(b0, np.float32),
        np.asarray(W1, np.float32), np.asarray(b1, np.float32),
        np.asarray(W2, np.float32),
    )
    if "nc" not in _CACHE:
        _CACHE["nc"] = _build_bass()
    nc = _CACHE["nc"]
    res = run_bass_kernel_spmd(
        nc, in_maps, core_ids=list(range(N_CORES)), trace=_trace,
    )
    if _results is not None:
        _results.append(res)
    parts = [res.results[c]["out_mT"].astype(np.float32) for c in range(N_CORES)]
    full_T = np.concatenate(parts, axis=1)          # [32, E]
    # device output omits the b2 term: m += h_w @ b2.reshape(o, i).T
    b2r = np.asarray(b2, np.float32).reshape(D, D)  # [o, i]
    return np.ascontiguousarray(full_T.T) + h_w @ b2r.T


if __name__ == "__main__":
    import reference
    inputs = reference.setup_inputs()
    inputs = {k: np.asarray(v) for k, v in inputs.items()}
    expected = np.asarray(reference.reference(**inputs))
    actual = kernel(**inputs)
    err = np.abs(actual - expected)
    denom = np.abs(expected).max()
    print("max abs err:", err.max(), "rel err:", err.max() / denom)


# revision 17
# speedup vs baseline: 1.3669x; 1.3669x over previous
"""Trainium2 Bass kernel for nn_MessageFunction (gnn_message_passing).

Computes, per edge e:
    x  = relu(e_vw @ W0.T + b0)                 # [E, 128]
    x  = relu(x @ W1.T + b1)                    # [E, 128]
    eo = (x @ W2.T + b2).reshape(E, 32, 32)     # [E, o, i]
    m  = einsum('eoi,ei->eo', eo, h_w)          # [E, 32]

Sharding: pure edge parallelism across 8 NeuronCores (E/8 = 16384 edges per
core), NNet parameters replicated.

Per-core layout strategy (host-side pre-transposition, fp16 on-chip matmul
dtypes, fp32 PSUM accumulation), supertiles of 512 edges:
  - L0/L1 run feature-major (hidden on partitions, edges free); relu+bias
    evictions on the scalar engine.
  - L2 runs oi-major: 4 pairs of 128-row chunks; pair p covers o in
    [8p, 8p+8) with i split in halves between its two chunks (W2 columns
    pair-reordered on host). Each pair lands in one [128, 2, 512] PSUM tile.
  - The per-edge h_w multiply is a fused PSUM-evict+multiply
    (scalar_tensor_tensor) on the vector engine for half the pairs, and a
    scalar-engine copy-evict + 2x-mode tensor_mul for the other half.
  - The i-contraction uses PE column-tiling: pair p's two selection matmuls
    (one shared [128, 8] 0/1 matrix) accumulate into col-strip p
    (tile_position (0, 32p)) of a single [128, 512] PSUM tile, so the four
    strips run concurrently on disjoint 32-column groups of the PE array.
    Each o lives in exactly one strip row -> no cross-strip reduction.
  - The b2 term (sum_i b2[o,i] h_w[e,i]) is independent of the NNet and is
    added on the host (cheap [E,32]@[32,32] einsum) during unsharding.
"""

import os
import sys
from contextlib import ExitStack

import numpy as np

sys.path.insert(0, "/opt/trn_rl_repo")

import concourse.bass as bass
import concourse.tile as tile
from concourse import bacc, mybir
from concourse._compat import with_exitstack
from concourse.bass_utils import run_bass_kernel_spmd

E = 131072
N_CORES = 8
E_CORE = E // N_CORES          # 16384
TILE_E = 128                   # edges per PE tile (e-major matmul M)
SUPER = 4                      # tiles per supertile
SUPER_E = SUPER * TILE_E       # 512
N_SUPER = E_CORE // SUPER_E    # 32
HID = 128
EF = 16
D = 32                         # D_IN == D_OUT == 32
OI = D * D                     # 1024
N_CHUNK = OI // 128            # 8
N_PAIR = N_CHUNK // 2          # 4

F32 = mybir.dt.float32
F16 = mybir.dt.float16

# Pairs [0, N_STT) evict+multiply fused on the DVE; the rest use an ACT
# copy-evict followed by a 2x-mode DVE multiply. 2/2 balances DVE vs ACT.
N_STT = 2


@with_exitstack
def _edge_mlp_kernel(
    ctx: ExitStack,
    tc: "tile.TileContext",
    out_mT: bass.AP,      # [32, E_CORE] fp32, o-major output (no b2 term)
    ev_t: bass.AP,        # [N_SUPER, EF, SUPER_E] fp16  (e_vw transposed)
    hw8: bass.AP,         # [N_SUPER, 128, 2, SUPER_E] fp16 (h_w^T in pair layout)
    w0t: bass.AP,         # [EF, HID] fp16
    w1t: bass.AP,         # [HID, HID] fp16
    w2t: bass.AP,         # [HID, OI] fp16 (columns pair-reordered on host)
    scm: bass.AP,         # [128, 8] fp16 shared selection matrix
    b0: bass.AP,          # [HID, 1] fp32
    b1: bass.AP,          # [HID, 1] fp32
):
    nc = tc.nc
    Relu = mybir.ActivationFunctionType.Relu
    Copy = mybir.ActivationFunctionType.Copy

    const = ctx.enter_context(tc.tile_pool(name="const", bufs=1))
    sup = ctx.enter_context(tc.tile_pool(name="sup", bufs=6))
    ypool = ctx.enter_context(tc.tile_pool(name="y", bufs=4))
    opool = ctx.enter_context(tc.tile_pool(name="o", bufs=3))
    ps_x = ctx.enter_context(tc.tile_pool(name="psx", bufs=2, space="PSUM"))
    ps_eo = ctx.enter_context(tc.tile_pool(name="pseo", bufs=2, space="PSUM"))
    ps_p = ctx.enter_context(tc.tile_pool(name="psp", bufs=1, space="PSUM"))

    # --- load constants once ---
    c_w0 = const.tile([EF, HID], F16)
    nc.sync.dma_start(c_w0[:], w0t[:])
    c_w1 = const.tile([HID, HID], F16)
    nc.sync.dma_start(c_w1[:], w1t[:])
    c_w2 = const.tile([HID, OI], F16)
    nc.sync.dma_start(c_w2[:], w2t[:])
    c_sc = const.tile([128, 8], F16)
    nc.sync.dma_start(c_sc[:], scm[:])
    c_b0 = const.tile([HID, 1], F32)
    nc.sync.dma_start(c_b0[:], b0[:])
    c_b1 = const.tile([HID, 1], F32)
    nc.sync.dma_start(c_b1[:], b1[:])

    # PE warmup: ~4us of matmuls on zeroed data while the first supertile's
    # DMAs land, so the HAM clock-gate reaches 8/8 before the real matmuls
    # start. Results land in the partials PSUM slot that the first real
    # start=True matmul group overwrites.
    warm_in = const.tile([128, SUPER_E], F16, tag="warm")
    warm_ps = ps_p.tile([128, SUPER_E], F32, tag="pp")
    nc.gpsimd.memset(warm_in[:], 0.0)
    for _ in range(18):
        nc.tensor.matmul(warm_ps[:], warm_in[:, 0:128], warm_in[:])

    def emit_sel(s, pp, ys):
        # i-contraction for supertile s: accumulating selection matmuls
        # into col-strip p (rows l in [0, 8) hold o = 8p + l). Emitted one
        # supertile late so all four y tiles already exist -> the PE runs
        # the block as two 4-strip concurrent bursts on disjoint column
        # groups instead of eight serial matmuls.
        for j in range(2):
            for p in range(N_PAIR):
                nc.tensor.matmul(
                    pp[32 * p : 32 * p + 8, :], c_sc[:], ys[p][:, j, :],
                    start=(j == 0), stop=(j == 1),
                    tile_position=(0, 32 * p),
                )
        # evict partials and store the 8 useful rows of each strip
        ps = opool.tile([128, SUPER_E], F16, tag="ps")
        nc.scalar.activation(ps[:], pp[:], Copy)
        for p in range(N_PAIR):
            nc.sync.dma_start(
                out_mT[8 * p : 8 * p + 8, s * SUPER_E : (s + 1) * SUPER_E],
                ps[32 * p : 32 * p + 8, :],
            )

    pend = None  # (s, pp, ys) of the previous supertile
    for s in range(N_SUPER):
        ev = sup.tile([EF, SUPER_E], F16, tag="ev")
        nc.sync.dma_start(ev[:], ev_t[s])
        hw = sup.tile([128, 2, SUPER_E], F16, tag="hw")
        nc.sync.dma_start(hw[:], hw8[s])

        # L0: x1T[h, e] = sum_f W0T[f, h] * evT[f, e]
        x1p = ps_x.tile([HID, SUPER_E], F32, tag="xp")
        nc.tensor.matmul(x1p[:], c_w0[:], ev[:])
        x1s = sup.tile([HID, SUPER_E], F16, tag="x1s")
        nc.scalar.activation(x1s[:], x1p[:], Relu, bias=c_b0[:])

        # L1: x2T[h2, e] = sum_h W1T[h, h2] * x1T[h, e]
        x2p = ps_x.tile([HID, SUPER_E], F32, tag="xp")
        nc.tensor.matmul(x2p[:], c_w1[:], x1s[:])
        x2s = sup.tile([HID, SUPER_E], F16, tag="x2s")
        nc.scalar.activation(x2s[:], x2p[:], Relu, bias=c_b1[:])

        # L2 + h_w multiply, per chunk pair.
        pp = ps_p.tile([128, SUPER_E], F32, tag="pp")
        ys = []
        for p in range(N_PAIR):
            c0 = 2 * p
            eo = ps_eo.tile([128, 2, SUPER_E], F32, tag="eo")
            nc.tensor.matmul(eo[:, 0, :], c_w2[:, c0 * 128 : (c0 + 1) * 128], x2s[:])
            nc.tensor.matmul(eo[:, 1, :], c_w2[:, (c0 + 1) * 128 : (c0 + 2) * 128], x2s[:])
            yc = ypool.tile([128, 2, SUPER_E], F16, tag=f"y{p}")
            if p % 2 == 0:
                # fused evict+mult on DVE: y = eo * hw
                nc.vector.scalar_tensor_tensor(
                    yc[:], eo[:], 1.0, hw[:],
                    op0=mybir.AluOpType.mult, op1=mybir.AluOpType.mult,
                )
            else:
                # ACT evicts (fp32 psum -> fp16 sbuf), DVE multiplies at 2x
                eos = ypool.tile([128, 2, SUPER_E], F16, tag=f"eos{p % 2}")
                nc.scalar.activation(eos[:], eo[:], Copy)
                nc.vector.tensor_mul(yc[:], eos[:], hw[:])
            ys.append(yc)
            # the previous supertile's i-contraction slots in mid-L2, when
            # all four of its y tiles are long since produced
            if p == 1 and pend is not None:
                emit_sel(*pend)
                pend = None
        pend = (s, pp, ys)

    emit_sel(*pend)


def _build_bass():
    nc = bacc.Bacc("TRN2", target_bir_lowering=False, debug=False)
    d = {}
    d["ev_t"] = nc.dram_tensor("ev_t", [N_SUPER, EF, SUPER_E], F16, kind="ExternalInput")
    d["hw8"] = nc.dram_tensor("hw8", [N_SUPER, 128, 2, SUPER_E], F16, kind="ExternalInput")
    d["w0t"] = nc.dram_tensor("w0t", [EF, HID], F16, kind="ExternalInput")
    d["w1t"] = nc.dram_tensor("w1t", [HID, HID], F16, kind="ExternalInput")
    d["w2t"] = nc.dram_tensor("w2t", [HID, OI], F16, kind="ExternalInput")
    d["scm"] = nc.dram_tensor("scm", [128, 8], F16, kind="ExternalInput")
    d["b0"] = nc.dram_tensor("b0", [HID, 1], F32, kind="ExternalInput")
    d["b1"] = nc.dram_tensor("b1", [HID, 1], F32, kind="ExternalInput")
    out = nc.dram_tensor("out_mT", [D, E_CORE], F16, kind="ExternalOutput")

    with tile.TileContext(nc) as tc:
        _edge_mlp_kernel(
            tc,
            out.ap(),
            d["ev_t"].ap(), d["hw8"].ap(),
            d["w0t"].ap(), d["w1t"].ap(), d["w2t"].ap(),
            d["scm"].ap(),
            d["b0"].ap(), d["b1"].ap(),
        )
    nc.compile()
    return nc


def _prep_host_inputs(h_w, e_vw, W0, b0, W1, b1, W2):
    """Build per-core input maps (all numpy, cheap)."""
    # shared (replicated) parameters
    w0t = np.ascontiguousarray(W0.T).astype(np.float16)            # [16, 128]
    w1t = np.ascontiguousarray(W1.T).astype(np.float16)            # [128, 128]
    # W2 columns in pair layout: pair p covers o in [8p, 8p+8); its two
    # chunks take i in [0,16) and [16,32). Within a chunk, partition
    # index = (o - 8p)*16 + (i mod 16).
    cols = np.empty((OI,), np.int64)
    for p in range(4):
        for j in range(2):
            o = np.repeat(np.arange(8 * p, 8 * p + 8), 16)          # [128]
            i = np.tile(np.arange(16 * j, 16 * j + 16), 8)          # [128]
            cols[(2 * p + j) * 128 : (2 * p + j + 1) * 128] = o * D + i
    w2t = np.ascontiguousarray(W2.T[:, cols]).astype(np.float16)    # [128, 1024]
    b0c = np.ascontiguousarray(b0.reshape(HID, 1)).astype(np.float32)
    b1c = np.ascontiguousarray(b1.reshape(HID, 1)).astype(np.float32)
    # shared selection matrix: scm[q, l] = 1 iff l == q // 16
    scm = np.zeros((128, 8), np.float16)
    q = np.arange(128)
    scm[q, q // 16] = 1.0

    in_maps = []
    for core in range(N_CORES):
        sl = slice(core * E_CORE, (core + 1) * E_CORE)
        ev_c = e_vw[sl]                                             # [16384, 16]
        hw_c = h_w[sl]                                              # [16384, 32]
        # ev_t[s, f, t*128+e] = ev_c[s*512 + t*128 + e, f]
        ev_t = np.ascontiguousarray(
            ev_c.reshape(N_SUPER, SUPER_E, EF).transpose(0, 2, 1)
        ).astype(np.float16)
        hw_t = hw_c.reshape(N_SUPER, SUPER_E, D).transpose(0, 2, 1)  # [Ns, 32, 512]
        # hw8[s, q, j, e] = hwT[s, 16*j + q%16, e]
        hw8 = np.empty((N_SUPER, 128, 2, SUPER_E), np.float16)
        qm = np.arange(128) % 16
        hw8[:, :, 0, :] = hw_t[:, qm, :]
        hw8[:, :, 1, :] = hw_t[:, 16 + qm, :]
        in_maps.append({
            "ev_t": ev_t, "hw8": hw8,
            "w0t": w0t, "w1t": w1t, "w2t": w2t,
            "scm": scm, "b0": b0c, "b1": b1c,
        })
    return in_maps


_CACHE = {}


def kernel(h_v, h_w, e_vw, W0, b0, W1, b1, W2, b2, _trace=False, _results=None):
    # h_v is unused by the reference computation (only its trailing dim of 1
    # matters there); the message depends on h_w, e_vw and the NNet params.
    del h_v
    h_w = np.asarray(h_w, np.float32)
    in_maps = _prep_host_inputs(
        h_w, np.asarray(e_vw, np.float32),
        np.asarray(W0, np.float32), np.asarray(b0, np.float32),
        np.asarray(W1, np.float32), np.asarray(b1, np.float32),
        np.asarray(W2, np.float32),
    )
    if "nc" not in _CACHE:
        _CACHE["nc"] = _build_bass()
    nc = _CACHE["nc"]
    res = run_bass_kernel_spmd(
        nc, in_maps, core_ids=list(range(N_CORES)), trace=_trace,
    )
    if _results is not None:
        _results.append(res)
    parts = [res.results[c]["out_mT"].astype(np.float32) for c in range(N_CORES)]
    full_T = np.concatenate(parts, axis=1)          # [32, E]
    # device output omits the b2 term: m += h_w @ b2.reshape(o, i).T
    b2r = np.asarray(b2, np.float32).reshape(D, D)  # [o, i]
    return np.ascontiguousarray(full_T.T) + h_w @ b2r.T


if __name__ == "__main__":
    import reference
    inputs = reference.setup_inputs()
    inputs = {k: np.asarray(v) for k, v in inputs.items()}
    expected = np.asarray(reference.reference(**inputs))
    actual = kernel(**inputs)
    err = np.abs(actual - expected)
    denom = np.abs(expected).max()
    print("max abs err:", err.max(), "rel err:", err.max() / denom)
